# revision 18
# baseline (speedup 1.0000x reference)
"""Trainium2 Bass kernel for nn_CrossAttention_47502338294587.

Math: the reference cross-attention has a single KV position broadcast over
all T query positions.  Softmax over a row of identical logits is uniform,
so attention output == v for every query, and the whole module collapses to

    out[b, t, :] = (visual_features[b] @ Wv + bv) @ Wp + bp      (for all t)

independent of x / Wq / Wk.  The device computes the two projections and
broadcasts the per-batch row over the T axis; the host only does input
layout prep and shard re-assembly (pure data movement, no arithmetic).

Sharding: tensor-parallel over the output channel dim C — core i computes
and writes out[:, :, i*128:(i+1)*128] (it loads full Wv but only its column
shard of Wp / bp).  With C-sharding, a core's whole output shard is one
[128, B*128] tile replicated over the 8 t-chunks, so the T-broadcast is a
single selector matmul + one replicated DMA.

Per-core structure:
  mm1:   vv = vf @ Wv          stationary vf^T chunks, moving Wv (N=512)
         + bv fused into the PSUM->SBUF copy (DVE tensor_add)
  tr:    vv^T chunks via PE transpose
  mm2:   row_sh = vv @ Wp[:,ci] (+ bp[ci] fused into copy)
  bcast: rhs4[k, b*128+c] = row_sh[k,c]*(k==b)  (DVE), then
         bc[t, (b,c)] = ones^T @ rhs4 (one matmul),
         one DMA with a step-0 replicated source writes all 8 t-chunks
"""

import os
import sys

import numpy as np

for _p in ("/opt/trn_rl_repo",):
    if _p not in sys.path and os.path.isdir(_p):
        sys.path.insert(0, _p)

B, T, C = 4, 1024, 1024
N_CORES = 8
CSH = C // N_CORES  # 128, C-shard per core
KC = C // 128  # 8 contraction chunks

_BUILT = None


def build_nc():
    """Build + compile the Bass program (one NeuronCore's SPMD body)."""
    import concourse.bass as bass
    import concourse.mybir as mybir
    import concourse.tile as tile
    from concourse import bacc
    from concourse.bass import ts

    f32 = mybir.dt.float32
    nc = bacc.Bacc("TRN2", target_bir_lowering=False, debug=False)

    wv = nc.dram_tensor("wv", [C, C], f32, kind="ExternalInput")
    # host pre-packs these into the exact SBUF layouts (pure layout prep):
    wp_p = nc.dram_tensor("wp_p", [128, KC * CSH], f32, kind="ExternalInput")
    vft_p = nc.dram_tensor("vft_p", [128, KC * B], f32, kind="ExternalInput")
    bv4 = nc.dram_tensor("bv4", [B, C], f32, kind="ExternalInput")
    bp4sel = nc.dram_tensor("bp4sel", [B, B * CSH], f32, kind="ExternalInput")
    # out[t, b, c_local]; host re-assembles full[b, t, ci] = out[t, b, :]
    out = nc.dram_tensor("out", [T, B, CSH], f32, kind="ExternalOutput")

    def band_select(ap, mult, width):
        """keep 1.0 inside the band 0 <= y - mult*k <= width-1, else 0."""
        nc.gpsimd.memset(ap, 1.0)
        nc.gpsimd.affine_select(
            out=ap, in_=ap, compare_op=mybir.AluOpType.is_ge, fill=0.0,
            base=0, pattern=[[1, ap.shape[-1]]], channel_multiplier=-mult,
        )
        nc.gpsimd.affine_select(
            out=ap, in_=ap, compare_op=mybir.AluOpType.is_ge, fill=0.0,
            base=width - 1, pattern=[[-1, ap.shape[-1]]], channel_multiplier=mult,
        )

    with tile.TileContext(nc) as tc:
        with tc.tile_pool(name="sb", bufs=1) as sb:
            # ---- SBUF tiles -------------------------------------------------
            wv_t = [sb.tile([128, C], f32, name=f"wv{k}", tag=f"wv{k}") for k in range(KC)]
            wp_t = sb.tile([128, KC, CSH], f32, tag="wp_t")
            vft_t = sb.tile([128, KC, B], f32, tag="vft")
            bv4_t = sb.tile([B, C], f32, tag="bv4")
            bp4sel_t = sb.tile([B, B * CSH], f32, tag="bp4sel")
            ones_bp = sb.tile([B, 128], f32, tag="ones_bp")
            # sel[k, b*128 + c] = (k == b)
            sel_t = sb.tile([B, B * 128], f32, tag="sel")
            ident_t = sb.tile([B, B], f32, tag="ident")
            vv_sb = sb.tile([B, C], f32, tag="vv_sb")
            vvt_t = sb.tile([128, KC * B], f32, tag="vvt")
            rhs4_t = sb.tile([B, B * CSH], f32, tag="rhs4")
            bc_t = sb.tile([128, B * CSH], f32, tag="bc")

            nc.vector.memset(ones_bp[:], 1.0)
            band_select(sel_t[:], 128, 128)
            band_select(ident_t[:], 1, 1)

            # ---- DMA in (first mm1 dependency first) ------------------------
            nc.scalar.dma_start(vft_t[:], vft_p.rearrange("p (k b) -> p k b", b=B))
            nc.scalar.dma_start(bv4_t[:], bv4[:, :])
            nc.scalar.dma_start(bp4sel_t[:], bp4sel[:, :])
            nc.scalar.dma_start(wp_t[:], wp_p.rearrange("p (k c) -> p k c", c=CSH))
            for k in range(KC):
                nc.sync.dma_start(wv_t[k][:], wv[ts(k, 128), :])

            # ---- mm1: vv[b, n] = sum_k vf[b, k] Wv[k, n]  (+bv via DVE) -----
            with tc.tile_pool(name="pv", bufs=2, space="PSUM") as pv:
                psum_vv = [pv.tile([B, 512], f32, name=f"pvv{h}", tag=f"pvv{h}") for h in range(2)]
                for k in range(KC):
                    for h in range(2):
                        nc.tensor.matmul(
                            psum_vv[h][:],
                            vft_t[:, k, :],
                            wv_t[k][:, ts(h, 512)],
                            start=(k == 0),
                            stop=(k == KC - 1),
                        )
                for h in range(2):
                    nc.vector.tensor_add(
                        vv_sb[0:B, ts(h, 512)], psum_vv[h][:], bv4_t[0:B, ts(h, 512)]
                    )

            # ---- transpose vv -> vv^T chunks [128, B] -----------------------
            with tc.tile_pool(name="pt", bufs=4, space="PSUM") as pt:
                for k in range(KC):
                    psum_vvt = pt.tile([128, B], f32, tag="pvt")
                    nc.tensor.transpose(
                        psum_vvt[:], vv_sb[0:B, ts(k, 128)], ident_t[0:B, 0:B]
                    )
                    nc.vector.tensor_copy(vvt_t[:, ts(k, B)], psum_vvt[:])

            # ---- mm2: row_sh = vv @ Wp[:,ci]  (+bp via DVE) -----------------
            with (
                tc.tile_pool(name="pr", bufs=1, space="PSUM") as pr,
                tc.tile_pool(name="pb", bufs=1, space="PSUM") as pb,
            ):
                psum_row = pr.tile([B, CSH], f32, tag="pr")
                for k in range(KC):
                    nc.tensor.matmul(
                        psum_row[:],
                        vvt_t[:, ts(k, B)],
                        wp_t[:, k, :],
                        start=(k == 0),
                        stop=(k == KC - 1),
                    )
                # ---- broadcast: one [128, B*CSH] tile == whole shard --------
                # rhs4 = rep4(psum_row) * sel + bp4sel   (bp fused via host-packed
                # block-diagonal bp4sel; rep4 = step-0 replicated AP)
                pra = psum_row[:]
                prep = bass.AP(
                    pra.tensor, pra.offset, [list(pra.ap[0]), [0, B], list(pra.ap[1])]
                )
                nc.vector.tensor_mul(
                    rhs4_t[:].rearrange("p (q f) -> p q f", q=B),
                    prep,
                    sel_t[:].rearrange("p (q f) -> p q f", q=B),
                )
                nc.vector.tensor_add(rhs4_t[:], rhs4_t[:], bp4sel_t[:])
                psum_bc = pb.tile([128, B * CSH], f32, tag="pb")
                nc.tensor.matmul(
                    psum_bc[:],
                    ones_bp[0:B, :],
                    rhs4_t[0:B, :],
                    start=True,
                    stop=True,
                )
                # split copy + replicated out-DMA into b-halves on separate
                # queues so the first half's write starts earlier
                half = B * CSH // 2
                out_v = out.rearrange("(q p) b c -> p q (b c)", p=128)
                for i, eng in ((0, nc.sync), (1, nc.scalar)):
                    nc.vector.tensor_copy(
                        bc_t[:, i * half : (i + 1) * half],
                        psum_bc[:, i * half : (i + 1) * half],
                    )
                    ap = bc_t[:, i * half : (i + 1) * half]
                    rep = bass.AP(
                        ap.tensor, ap.offset, [list(ap.ap[0]), [0, KC], list(ap.ap[1])]
                    )
                    eng.dma_start(out_v[:, :, i * half : (i + 1) * half], rep)

    nc.compile()
    return nc


def _get_built():
    global _BUILT
    if _BUILT is None:
        _BUILT = build_nc()
    return _BUILT


def make_in_maps(inputs):
    vf = np.asarray(inputs["visual_features"], np.float32)
    wv = np.ascontiguousarray(np.asarray(inputs["Wv"], np.float32))
    wp = np.asarray(inputs["Wp"], np.float32)
    bv = np.asarray(inputs["bv"], np.float32)
    bp = np.asarray(inputs["bp"], np.float32)
    # vft_p[p, k*B + b] = vf[b, k*128 + p]
    vft_p = np.ascontiguousarray(
        vf.T.reshape(KC, 128, B).transpose(1, 0, 2).reshape(128, KC * B)
    )
    bv4 = np.ascontiguousarray(np.broadcast_to(bv[None, :], (B, C)))
    maps = []
    for i in range(N_CORES):
        ci = slice(i * CSH, (i + 1) * CSH)
        # wp_p[p, k*CSH + c] = Wp[k*128 + p, ci_c]
        wp_p = np.ascontiguousarray(
            wp[:, ci].reshape(KC, 128, CSH).transpose(1, 0, 2).reshape(128, KC * CSH)
        )
        bp4sel = np.zeros((B, B * CSH), np.float32)
        for b in range(B):
            bp4sel[b, b * CSH : (b + 1) * CSH] = bp[ci]
        maps.append(
            {"wv": wv, "wp_p": wp_p, "vft_p": vft_p, "bv4": bv4, "bp4sel": bp4sel}
        )
    return maps


def run(inputs, trace=False, **kw):
    from concourse.bass_utils import run_bass_kernel_spmd

    nc = _get_built()
    res = run_bass_kernel_spmd(
        nc,
        make_in_maps(inputs),
        core_ids=list(range(N_CORES)),
        trace=trace,
        **kw,
    )
    full = np.empty((B, T, C), np.float32)
    for i, r in enumerate(res.results):
        full[:, :, i * CSH : (i + 1) * CSH] = r["out"].transpose(1, 0, 2)
    return full, res


def kernel(**inputs) -> np.ndarray:
    full, _ = run(inputs, trace=False)
    return full


# revision 21
# speedup vs baseline: 1.0506x; 1.0506x over previous
"""Trainium2 Bass kernel for nn_CrossAttention_47502338294587.

Math: the reference cross-attention has a single KV position broadcast over
all T query positions.  Softmax over a row of identical logits is uniform,
so attention output == v for every query, and the whole module collapses to

    out[b, t, :] = (visual_features[b] @ Wv + bv) @ Wp + bp      (for all t)

independent of x / Wq / Wk.  The device computes the two projections and
broadcasts the per-batch row over the T axis; the host only does input
layout prep and shard re-assembly (pure data movement, no arithmetic).

Sharding: tensor-parallel over the output channel dim C — core i computes
and writes out[:, :, i*128:(i+1)*128] (it loads full Wv but only its column
shard of Wp / bp).  With C-sharding, a core's whole output shard is one
[128, B*128] tile replicated over the 8 t-chunks, so the T-broadcast is a
single selector matmul + one replicated DMA.

Per-core structure:
  mm1:   vv = vf @ Wv          stationary vf^T chunks, moving Wv (N=512)
         + bv fused into the PSUM->SBUF copy (DVE tensor_add)
  tr:    vv^T chunks via PE transpose
  mm2:   row_sh = vv @ Wp[:,ci] (+ bp[ci] fused into copy)
  bcast: rhs4[k, b*128+c] = row_sh[k,c]*(k==b)  (DVE), then
         bc[t, (b,c)] = ones^T @ rhs4 (one matmul),
         one DMA with a step-0 replicated source writes all 8 t-chunks
"""

import os
import sys

import numpy as np

for _p in ("/opt/trn_rl_repo",):
    if _p not in sys.path and os.path.isdir(_p):
        sys.path.insert(0, _p)

B, T, C = 4, 1024, 1024
N_CORES = 8
CSH = C // N_CORES  # 128, C-shard per core
KC = C // 128  # 8 contraction chunks

_BUILT = None


def build_nc():
    """Build + compile the Bass program (one NeuronCore's SPMD body)."""
    import concourse.bass as bass
    import concourse.mybir as mybir
    import concourse.tile as tile
    from concourse import bacc
    from concourse.bass import ts

    f32 = mybir.dt.float32
    nc = bacc.Bacc("TRN2", target_bir_lowering=False, debug=False)

    wv = nc.dram_tensor("wv", [C, C], f32, kind="ExternalInput")
    # host pre-packs these into the exact SBUF layouts (pure layout prep):
    wp_p = nc.dram_tensor("wp_p", [128, KC * CSH], f32, kind="ExternalInput")
    vft_p = nc.dram_tensor("vft_p", [128, KC * B], f32, kind="ExternalInput")
    bv4 = nc.dram_tensor("bv4", [B, C], f32, kind="ExternalInput")
    bp4sel = nc.dram_tensor("bp4sel", [B, B * CSH], f32, kind="ExternalInput")
    # out[t, b, c_local]; host re-assembles full[b, t, ci] = out[t, b, :]
    out = nc.dram_tensor("out", [T, B, CSH], f32, kind="ExternalOutput")

    def band_select(ap, mult, width):
        """keep 1.0 inside the band 0 <= y - mult*k <= width-1, else 0."""
        nc.gpsimd.memset(ap, 1.0)
        nc.gpsimd.affine_select(
            out=ap, in_=ap, compare_op=mybir.AluOpType.is_ge, fill=0.0,
            base=0, pattern=[[1, ap.shape[-1]]], channel_multiplier=-mult,
        )
        nc.gpsimd.affine_select(
            out=ap, in_=ap, compare_op=mybir.AluOpType.is_ge, fill=0.0,
            base=width - 1, pattern=[[-1, ap.shape[-1]]], channel_multiplier=mult,
        )

    with tile.TileContext(nc) as tc:
        with tc.tile_pool(name="sb", bufs=1) as sb:
            # ---- SBUF tiles -------------------------------------------------
            wv_t = [sb.tile([128, C], f32, name=f"wv{k}", tag=f"wv{k}") for k in range(KC)]
            wp_t = sb.tile([128, KC, CSH], f32, tag="wp_t")
            vft_t = sb.tile([128, KC, B], f32, tag="vft")
            bv4_t = sb.tile([B, C], f32, tag="bv4")
            bp4sel_t = sb.tile([B, B * CSH], f32, tag="bp4sel")
            ones_bp = sb.tile([B, 128], f32, tag="ones_bp")
            # sel[k, b*128 + c] = (k == b)
            sel_t = sb.tile([B, B * 128], f32, tag="sel")
            ident_t = sb.tile([B, B], f32, tag="ident")
            vv_sb = sb.tile([B, C], f32, tag="vv_sb")
            vvt_t = sb.tile([128, KC * B], f32, tag="vvt")
            rhs4_t = sb.tile([B, B * CSH], f32, tag="rhs4")
            bc_t = sb.tile([128, B * CSH], f32, tag="bc")

            nc.vector.memset(ones_bp[:], 1.0)
            band_select(sel_t[:], 128, 128)
            band_select(ident_t[:], 1, 1)

            # ---- DMA in (first mm1 dependency first) ------------------------
            nc.scalar.dma_start(vft_t[:], vft_p.rearrange("p (k b) -> p k b", b=B))
            nc.scalar.dma_start(bv4_t[:], bv4[:, :])
            nc.scalar.dma_start(bp4sel_t[:], bp4sel[:, :])
            nc.scalar.dma_start(wp_t[:], wp_p.rearrange("p (k c) -> p k c", c=CSH))
            for k in range(KC):
                nc.sync.dma_start(wv_t[k][:], wv[ts(k, 128), :])

            # ---- mm1: vv[b, n] = sum_k vf[b, k] Wv[k, n]  (+bv via DVE) -----
            with tc.tile_pool(name="pv", bufs=2, space="PSUM") as pv:
                psum_vv = [pv.tile([B, 512], f32, name=f"pvv{h}", tag=f"pvv{h}") for h in range(2)]
                for k in range(KC):
                    for h in range(2):
                        nc.tensor.matmul(
                            psum_vv[h][:],
                            vft_t[:, k, :],
                            wv_t[k][:, ts(h, 512)],
                            start=(k == 0),
                            stop=(k == KC - 1),
                        )
                for h in range(2):
                    nc.vector.tensor_add(
                        vv_sb[0:B, ts(h, 512)], psum_vv[h][:], bv4_t[0:B, ts(h, 512)]
                    )

            # ---- transpose vv -> vv^T chunks [128, B] -----------------------
            with tc.tile_pool(name="pt", bufs=4, space="PSUM") as pt:
                for k in range(KC):
                    psum_vvt = pt.tile([128, B], f32, tag="pvt")
                    nc.tensor.transpose(
                        psum_vvt[:], vv_sb[0:B, ts(k, 128)], ident_t[0:B, 0:B]
                    )
                    nc.vector.tensor_copy(vvt_t[:, ts(k, B)], psum_vvt[:])

            # ---- mm2: row_sh = vv @ Wp[:,ci]  (+bp via DVE) -----------------
            with (
                tc.tile_pool(name="pr", bufs=1, space="PSUM") as pr,
                tc.tile_pool(name="pb", bufs=1, space="PSUM") as pb,
            ):
                psum_row = pr.tile([B, CSH], f32, tag="pr")
                for k in range(KC):
                    nc.tensor.matmul(
                        psum_row[:],
                        vvt_t[:, ts(k, B)],
                        wp_t[:, k, :],
                        start=(k == 0),
                        stop=(k == KC - 1),
                    )
                # ---- broadcast: one [128, B*CSH] tile == whole shard --------
                # rhs4 = rep4(psum_row) * sel + bp4sel   (bp fused via host-packed
                # block-diagonal bp4sel; rep4 = step-0 replicated AP)
                pra = psum_row[:]
                prep = bass.AP(
                    pra.tensor, pra.offset, [list(pra.ap[0]), [0, B], list(pra.ap[1])]
                )
                nc.vector.tensor_mul(
                    rhs4_t[:].rearrange("p (q f) -> p q f", q=B),
                    prep,
                    sel_t[:].rearrange("p (q f) -> p q f", q=B),
                )
                nc.vector.tensor_add(rhs4_t[:], rhs4_t[:], bp4sel_t[:])
                psum_bc = pb.tile([128, B * CSH], f32, tag="pb")
                nc.tensor.matmul(
                    psum_bc[:],
                    ones_bp[0:B, :],
                    rhs4_t[0:B, :],
                    start=True,
                    stop=True,
                )
                # split copy + replicated out-DMA into b-halves on separate
                # queues so the first half's write starts earlier
                half = B * CSH // 2
                out_v = out.rearrange("(q p) b c -> p q (b c)", p=128)
                for i, eng in ((0, nc.sync), (1, nc.scalar)):
                    nc.vector.tensor_copy(
                        bc_t[:, i * half : (i + 1) * half],
                        psum_bc[:, i * half : (i + 1) * half],
                    )
                    ap = bc_t[:, i * half : (i + 1) * half]
                    rep = bass.AP(
                        ap.tensor, ap.offset, [list(ap.ap[0]), [0, KC], list(ap.ap[1])]
                    )
                    eng.dma_start(out_v[:, :, i * half : (i + 1) * half], rep)

    nc.compile()
    return nc


def _get_built():
    global _BUILT
    if _BUILT is None:
        _BUILT = build_nc()
    return _BUILT


def make_in_maps(inputs):
    vf = np.asarray(inputs["visual_features"], np.float32)
    wv = np.ascontiguousarray(np.asarray(inputs["Wv"], np.float32))
    wp = np.asarray(inputs["Wp"], np.float32)
    bv = np.asarray(inputs["bv"], np.float32)
    bp = np.asarray(inputs["bp"], np.float32)
    # vft_p[p, k*B + b] = vf[b, k*128 + p]
    vft_p = np.ascontiguousarray(
        vf.T.reshape(KC, 128, B).transpose(1, 0, 2).reshape(128, KC * B)
    )
    bv4 = np.ascontiguousarray(np.broadcast_to(bv[None, :], (B, C)))
    maps = []
    for i in range(N_CORES):
        ci = slice(i * CSH, (i + 1) * CSH)
        # wp_p[p, k*CSH + c] = Wp[k*128 + p, ci_c]
        wp_p = np.ascontiguousarray(
            wp[:, ci].reshape(KC, 128, CSH).transpose(1, 0, 2).reshape(128, KC * CSH)
        )
        bp4sel = np.zeros((B, B * CSH), np.float32)
        for b in range(B):
            bp4sel[b, b * CSH : (b + 1) * CSH] = bp[ci]
        maps.append(
            {"wv": wv, "wp_p": wp_p, "vft_p": vft_p, "bv4": bv4, "bp4sel": bp4sel}
        )
    return maps


def run(inputs, trace=False, **kw):
    from concourse.bass_utils import run_bass_kernel_spmd

    nc = _get_built()
    res = run_bass_kernel_spmd(
        nc,
        make_in_maps(inputs),
        core_ids=list(range(N_CORES)),
        trace=trace,
        **kw,
    )
    full = np.empty((B, T, C), np.float32)
    for i, r in enumerate(res.results):
        full[:, :, i * CSH : (i + 1) * CSH] = r["out"].transpose(1, 0, 2)
    return full, res


def kernel(**inputs) -> np.ndarray:
    full, _ = run(inputs, trace=False)
    return full
